# revision 15
# baseline (speedup 1.0000x reference)
"""Trainium2 Bass kernel for PVT-style MHSA with spatial reduction.

Problem (hardcoded): B=4, C=384, H=W=64, NH=8 heads, HD=48, SR=2.
  q = Wq@x;  xsr = conv2x2s2(x, Wsr)+bsr;  k = (Wk@xsr + pos)*scale;  v = Wv@xsr
  attn = softmax(q^T k);  out = Wp@(v attn) + bp

Sharding: 8 cores = (batch b, query-half s).  Each core computes the full
conv/k/v for its batch (duplicated across the 2 cores of a batch) and
attention + projection for its 2048 queries.  No collectives.

Device notes (v3):
  - heads padded 48 -> 64 channels; head-pair hp occupies one 128-row tile.
  - attention computed transposed: attnT[m, n] = sum_d k[d, m] q[d, n]; the
    key-axis softmax reduction rides the AV matmul via an all-ones column in
    v^T at head-local col 32 (rowsums on 32-aligned PSUM partitions 32/96,
    extracted by one DVE + one ACT copy); QK^T pairs row-packed, AV pairs
    col-packed in the PE array via tile_position.
  - exp is split: ~40% of key-tiles computed on DVE via a Schraudolph
    bit-trick (x*K+B -> int16, bitcast fp16), the rest on ACT's spline exp.
    This keeps the PE array dense (no HAM re-throttle) and halves ACT time.
  - softmax reciprocal: rowsum pair [2,512] scatter-DMA'd to [64,16],
    reciprocal there (16 el/lane instead of 512), gathered to DRAM and
    partition-broadcast back.  ~35x less DVE time than recip on [1,512].
  - e / v^T / o / Wp are fp16 (more mantissa than bf16 - frees error budget
    for the trick); q/k and phase A stay float32r.
  - x is loaded once ([128,2,CT,2048], own query-half first per core; pos
    key-order swapped to match), q reads slices of it directly.
"""

import threading

import numpy as np

import concourse.bass as bass
import concourse.mybir as mybir
import concourse.tile as tile
from concourse import bacc
from concourse.bass import ts
from concourse.bass_utils import run_bass_kernel_spmd

B, C, H, W = 4, 384, 64, 64
NH, HD, SR = 8, 48, 2
SCALE = HD ** -0.5
Hs, Ws = H // SR, W // SR
NK = Hs * Ws            # 1024 keys
N = H * W               # 4096 queries / batch
NQ = N // 2             # 2048 queries / core
CT = C // 128           # 3 c-tiles
HP = NH // 2            # 4 head-pair tiles
NB = NQ // 512          # 4 query blocks / core
MT = NK // 128          # 8 key tiles

F32 = mybir.dt.float32
F32R = mybir.dt.float32r
F16 = mybir.dt.float16
I16 = mybir.dt.int16
AF = mybir.ActivationFunctionType
ALU = mybir.AluOpType

# fp16 Schraudolph: i16 = x*(1024/ln2) + (15*1024 - 1024*c - 0.5), bitcast f16
TRICK_K = 1024.0 / float(np.log(2.0))
TRICK_B = 15.0 * 1024.0 - 1024.0 * 0.043677448 - 0.5

DEFAULT_CFG = dict(
    psa_bufs=6, qk_bufs=2, av_bufs=3, pr_bufs=1, e_bufs=2, r_bufs=3, dr_bufs=3,
    dve_mi=2,       # key-tiles per (nb,hp) exp'd on DVE (of MT=8)
    dve_extra=2,    # if >0: every dve_extra-th (nb,hp) group gets one more
    warm_qk=(1, 3, 5),   # dummy warm-keeper MMs after these QK tiles
    warm_av=(2, 5),      # ... and after these AV tiles
)


def build_program(**cfg):
    cfg = {**DEFAULT_CFG, **cfg}
    nc = bacc.Bacc(None, target_bir_lowering=False)

    xf = nc.dram_tensor("xf", [128, 2, CT, N // 2], F32R, kind="ExternalInput")
    wq = nc.dram_tensor("wq", [128, CT, 512], F32R, kind="ExternalInput")
    wk = nc.dram_tensor("wk", [128, CT, 512], F32R, kind="ExternalInput")
    wv = nc.dram_tensor("wv", [128, CT, 512], F32R, kind="ExternalInput")
    wsr = nc.dram_tensor("wsr", [128, 12, C], F32R, kind="ExternalInput")
    wp = nc.dram_tensor("wp", [128, 4, C], F16, kind="ExternalInput")
    pos = nc.dram_tensor("pos", [128, HP, NK], F32, kind="ExternalInput")
    bsr = nc.dram_tensor("bsr", [128, CT], F32, kind="ExternalInput")
    bp = nc.dram_tensor("bp", [128, CT], F32, kind="ExternalInput")
    out = nc.dram_tensor("out", [128, CT, NQ], F32, kind="ExternalOutput")

    with tile.TileContext(nc) as tc:
        with (
            tc.tile_pool(name="constp", bufs=1) as constp,
            tc.tile_pool(name="actp", bufs=1) as actp,
        ):
            wk_sb = constp.tile([128, CT, 512], F32R, name="wk_sb")
            wv_sb = constp.tile([128, CT, 512], F32R, name="wv_sb")
            wp_sb = constp.tile([128, 4, C], F16, name="wp_sb")
            bsr_sb = constp.tile([128, CT], F32, name="bsr_sb")
            bp_sb = constp.tile([128, CT], F32, name="bp_sb")

            q_sb = actp.tile([128, HP, NQ], F32R, name="q_sb")
            k_sb = actp.tile([128, HP, NK], F32R, name="k_sb")
            vt_sb = actp.tile([128, MT, 512], F16, name="vt_sb")

            # ---- phase A: conv + projections -----------------------------
            with (
                tc.tile_pool(name="aload", bufs=1) as aload,
                tc.tile_pool(name="psA", bufs=cfg["psa_bufs"], space="PSUM") as psA,
            ):
                xf_sb = aload.tile([128, 2, CT, N // 2], F32R, name="xf_sb")
                wq_sb = aload.tile([128, CT, 512], F32R, name="wq_sb")
                wsr_sb = aload.tile([128, 12, C], F32R, name="wsr_sb")
                pos_sb = aload.tile([128, HP, NK], F32, name="pos_sb")
                xsr_sb = aload.tile([128, CT, NK], F32R, name="xsr_sb")

                # ACT HWDGE ring: weights/bias/pos (ACT is idle until exps)
                nc.scalar.dma_start(wsr_sb[:], wsr[:])
                nc.scalar.dma_start(wk_sb[:], wk[:])
                nc.scalar.dma_start(wq_sb[:], wq[:])
                nc.scalar.dma_start(bsr_sb[:], bsr[:])
                nc.scalar.dma_start(pos_sb[:], pos[:])
                nc.scalar.dma_start(wv_sb[:], wv[:])
                nc.scalar.dma_start(wp_sb[:], wp[:])
                nc.scalar.dma_start(bp_sb[:], bp[:])
                # SP HWDGE ring: activations, ordered by first use
                nc.sync.dma_start(xf_sb[:, 0], xf[:, 0])
                nc.sync.dma_start(xf_sb[:, 1], xf[:, 1])

                def emit_conv(mb):
                    for ot in range(CT):
                        p = psA.tile([128, 512], F32, name="pa", tag="pa")
                        n_mm = 0
                        for didj in range(4):
                            di, dj = didj // 2, didj % 2
                            for ci in range(CT):
                                base = xf_sb[:]
                                rhs = bass.AP(
                                    tensor=base.tensor,
                                    offset=base.offset
                                    + mb * (CT * N // 2)
                                    + ci * (N // 2)
                                    + di * W
                                    + dj,
                                    ap=[base.ap[0], [2 * W, Hs // 2], [2, Ws]],
                                )
                                nc.tensor.matmul(
                                    p[:],
                                    wsr_sb[:, didj * CT + ci, ts(ot, 128)],
                                    rhs,
                                    start=(n_mm == 0),
                                    stop=(n_mm == 11),
                                )
                                n_mm += 1
                        nc.vector.tensor_scalar_add(
                            xsr_sb[:, ot, ts(mb, 512)], p[:], bsr_sb[:, ot : ot + 1]
                        )

                def emit_k(hp, mb):
                    p = psA.tile([128, 512], F32, name="pa", tag="pa")
                    for ci in range(CT):
                        nc.tensor.matmul(
                            p[:],
                            wk_sb[:, ci, ts(hp, 128)],
                            xsr_sb[:, ci, ts(mb, 512)],
                            start=(ci == 0),
                            stop=(ci == CT - 1),
                        )
                    nc.vector.tensor_add(
                        k_sb[:, hp, ts(mb, 512)], p[:], pos_sb[:, hp, ts(mb, 512)]
                    )

                def emit_q(ot, nb):
                    p = psA.tile([128, 512], F32, name="pa", tag="pa")
                    for ci in range(CT):
                        nc.tensor.matmul(
                            p[:],
                            wq_sb[:, ci, ts(ot, 128)],
                            xf_sb[:, 0, ci, ts(nb, 512)],
                            start=(ci == 0),
                            stop=(ci == CT - 1),
                        )
                    nc.vector.tensor_copy(q_sb[:, ot, ts(nb, 512)], p[:])

                def emit_vt(mi):
                    p = psA.tile([128, 512], F32, name="pa", tag="pa")
                    for ci in range(CT):
                        nc.tensor.matmul(
                            p[:],
                            xsr_sb[:, ci, ts(mi, 128)],
                            wv_sb[:, ci, :],
                            start=(ci == 0),
                            stop=(ci == CT - 1),
                        )
                    nc.vector.tensor_copy(vt_sb[:, mi, :], p[:])
                    base = vt_sb[:]
                    # ones at head-local col 32 -> rowsums on 32-aligned
                    # PSUM partitions 32 / 96 (engine APs need 32-aligned
                    # partition bases)
                    ones_ap = bass.AP(
                        tensor=base.tensor,
                        offset=base.offset + mi * 512 + 32,
                        ap=[base.ap[0], [64, NH]],
                    )
                    nc.gpsimd.memset(ones_ap, 1.0)

                # emission order shapes the schedule: unblock (hp0, nb0)
                # attention as early as possible
                emit_conv(0)
                for hp in range(HP):
                    emit_k(hp, 0)
                emit_q(0, 0)
                emit_conv(1)
                for hp in range(HP):
                    emit_k(hp, 1)
                for mi in range(4):
                    emit_vt(mi)
                for ot in range(1, HP):
                    emit_q(ot, 0)
                for mi in range(4, MT):
                    emit_vt(mi)
                for nb in range(1, NB):
                    for ot in range(HP):
                        emit_q(ot, nb)

            # ---- phase B: attention + projection -------------------------
            with (
                tc.tile_pool(name="bpool", bufs=1) as bpool,
                tc.tile_pool(name="epool", bufs=cfg["e_bufs"]) as epool,
                tc.tile_pool(name="rpool", bufs=cfg["r_bufs"]) as rpool,
                tc.tile_pool(name="drp", bufs=cfg["dr_bufs"], space="DRAM") as drp,
                tc.tile_pool(name="qkps", bufs=cfg["qk_bufs"], space="PSUM") as qkps,
                tc.tile_pool(name="avps", bufs=cfg["av_bufs"], space="PSUM") as avps,
                tc.tile_pool(name="prps", bufs=cfg["pr_bufs"], space="PSUM") as prps,
            ):
                o_sb = bpool.tile([128, HP, NQ], F16, name="o_sb")
                outp_sb = bpool.tile([128, CT, NQ], F32, name="outp_sb")

                # The normalize chain (rowsum extract -> lane-spread recip ->
                # DRAM bounce -> partition-broadcast -> multiply) has ~5us of
                # DMA latency.  Emitting it inline would block the engine
                # queues (in-order!) and starve the PE -> HAM re-throttle.
                # So it is software-pipelined across tile groups:
                #   front of group g:  norm-mul(g-2), proj if complete,
                #                      extract+scatter(g-1)
                #   mid of group g:    recip+gather+broadcast(g-1)
                groups = [(nb, hp) for nb in range(NB) for hp in range(HP)]
                state = {}

                def emit_proj(nb):
                    for ot in range(CT):
                        p = prps.tile([128, 512], F32, name="pp", tag="pp")
                        for dd in range(4):
                            nc.tensor.matmul(
                                p[:],
                                wp_sb[:, dd, ts(ot, 128)],
                                o_sb[:, dd, ts(nb, 512)],
                                start=(dd == 0),
                                stop=(dd == 3),
                            )
                        nc.vector.tensor_scalar_add(
                            outp_sb[:, ot, ts(nb, 512)],
                            p[:],
                            bp_sb[:, ot : ot + 1],
                        )
                    nc.sync.dma_start(
                        out[:, :, ts(nb, 512)], outp_sb[:, :, ts(nb, 512)]
                    )

                def emit_norm(s):
                    nc.vector.tensor_mul(
                        o_sb[:, s["hp"], ts(s["nb"], 512)], s["oav"][:], s["rb"][:]
                    )
                    if s["hp"] == HP - 1:
                        emit_proj(s["nb"])

                def emit_extract(s):
                    ex = rpool.tile([33, 512], F32, name="ex", tag="ex")
                    nc.vector.tensor_copy(ex[0:1, :], s["oav"][32:33, :])
                    nc.vector.tensor_copy(ex[32:33, :], s["oav"][96:97, :])
                    rs = rpool.tile([64, 16], F32, name="rs", tag="rs")
                    nc.gpsimd.dma_start(rs[0:32, :], ex[0:1, :])
                    nc.gpsimd.dma_start(rs[32:64, :], ex[32:33, :])
                    s["rs"] = rs

                def emit_recip_bcast(s):
                    rr = rpool.tile([64, 16], F32, name="rr", tag="rr")
                    nc.vector.reciprocal(out=rr[:], in_=s["rs"][:])
                    r2d = drp.tile([2, 512], F32, name="r2d", tag="r2d")
                    nc.gpsimd.dma_start(r2d[0:1], rr[0:32, :])
                    nc.gpsimd.dma_start(r2d[1:2], rr[32:64, :])
                    rb = rpool.tile([128, 512], F32, name="rb", tag="rb")
                    nc.sync.dma_start(
                        rb[0:64, :].unsqueeze(1),
                        r2d[0:1, :].partition_broadcast(64),
                    )
                    nc.sync.dma_start(
                        rb[64:128, :].unsqueeze(1),
                        r2d[1:2, :].partition_broadcast(64),
                    )
                    s["rb"] = rb

                def emit_warm():
                    # dependency-free matmul into the (idle) proj PSUM slot:
                    # fills PE micro-idle gaps so the HAM activity monitor
                    # keeps the PE array at 2.4 GHz (idle gaps re-throttle
                    # it to 1.2 GHz, doubling every matmul).
                    p = prps.tile([128, 512], F32, name="pp", tag="pp")
                    nc.tensor.matmul(
                        p[:], wp_sb[:, 0, 0:128], vt_sb[:, 0, :],
                        start=True, stop=True,
                    )

                for g, (nb, hp) in enumerate(groups):
                    if g >= 2:
                        emit_norm(state.pop(g - 2))
                    if g >= 1:
                        emit_extract(state[g - 1])

                    dve_n = cfg["dve_mi"] + (
                        1 if cfg["dve_extra"] and g % cfg["dve_extra"] == 0 else 0
                    )
                    e_sb = epool.tile([128, MT, 1024], F16, name="e", tag="e")
                    for mi in range(MT):
                        qk = qkps.tile([128, 1024], F32, name="qk", tag="qk")
                        nc.tensor.matmul(
                            qk[:, 0:512],
                            k_sb[0:64, hp, ts(mi, 128)],
                            q_sb[0:64, hp, ts(nb, 512)],
                            start=True,
                            stop=True,
                            tile_position=(0, 0),
                        )
                        nc.tensor.matmul(
                            qk[:, 512:1024],
                            k_sb[64:128, hp, ts(mi, 128)],
                            q_sb[64:128, hp, ts(nb, 512)],
                            start=True,
                            stop=True,
                            tile_position=(64, 0),
                        )
                        if mi >= MT - dve_n:
                            nc.vector.tensor_scalar(
                                out=e_sb[:, mi, :].bitcast(I16),
                                in0=qk[:],
                                scalar1=TRICK_K,
                                scalar2=TRICK_B,
                                op0=ALU.mult,
                                op1=ALU.add,
                            )
                        else:
                            nc.scalar.activation(
                                out=e_sb[:, mi, :], in_=qk[:], func=AF.Exp
                            )
                        if mi in cfg["warm_qk"]:
                            emit_warm()

                    if g >= 1:
                        emit_recip_bcast(state[g - 1])

                    oav = avps.tile([128, 512], F32, name="oav", tag="oav")
                    for mi in range(MT):
                        nc.tensor.matmul(
                            oav[0:64, :],
                            vt_sb[:, mi, 128 * hp : 128 * hp + 64],
                            e_sb[:, mi, 0:512],
                            start=(mi == 0),
                            stop=(mi == MT - 1),
                            tile_position=(0, 0),
                            skip_group_check=True,
                        )
                        nc.tensor.matmul(
                            oav[64:128, :],
                            vt_sb[:, mi, 128 * hp + 64 : 128 * (hp + 1)],
                            e_sb[:, mi, 512:1024],
                            start=(mi == 0),
                            stop=(mi == MT - 1),
                            tile_position=(0, 64),
                            skip_group_check=True,
                        )
                        if mi in cfg["warm_av"]:
                            emit_warm()
                    state[g] = {"nb": nb, "hp": hp, "oav": oav}

                # drain the pipeline
                ng = len(groups)
                emit_norm(state.pop(ng - 2))
                emit_extract(state[ng - 1])
                emit_recip_bcast(state[ng - 1])
                emit_norm(state.pop(ng - 1))

    nc.compile()
    return nc


def _pad_cols(w):
    """[C, C] weight -> [C, 512]: col 64h+j = w[48h+j, :] (j < 48)."""
    wt = np.zeros((C, NH * 64), np.float32)
    for h in range(NH):
        wt[:, 64 * h : 64 * h + HD] = w[HD * h : HD * (h + 1), :].T
    return wt


# v/proj head-local channel placement: ones column at local col 32 so the
# rowsum lands on a 32-aligned PSUM partition; channel d -> col d (d<32)
# else d+1
_VCOL = np.array([d if d < 32 else d + 1 for d in range(HD)])


def _vcol(h):
    return _VCOL


def _ctile(w, dt=np.float32):
    """[C, F] -> [128, CT, F] (partition-major c-tiles)."""
    return np.ascontiguousarray(
        w.reshape(CT, 128, -1).transpose(1, 0, 2)
    ).astype(dt)


def prep_inputs(inputs):
    x = np.ascontiguousarray(np.asarray(inputs["x"], np.float32))
    Wq = np.asarray(inputs["Wq"], np.float32)
    Wk = np.asarray(inputs["Wk"], np.float32)
    Wv = np.asarray(inputs["Wv"], np.float32)
    Wsr = np.asarray(inputs["Wsr"], np.float32)
    bsr = np.asarray(inputs["bsr"], np.float32)
    Wp = np.asarray(inputs["Wp"], np.float32)
    bp = np.asarray(inputs["bp"], np.float32)
    rel_h = np.asarray(inputs["rel_h"], np.float32)
    rel_w = np.asarray(inputs["rel_w"], np.float32)

    wq_t = _ctile(_pad_cols(Wq))
    wk_t = _ctile(_pad_cols(Wk) * SCALE)
    wv_pad = np.zeros((C, NH * 64), np.float32)
    for h in range(NH):
        wv_pad[:, 64 * h + _vcol(h)] = Wv[HD * h : HD * (h + 1), :].T
    wv_t = _ctile(wv_pad)
    # conv weights: rows ordered (di, dj, c) -> [128, 12, C] (didj, ci) tiles
    wsr_t = np.ascontiguousarray(
        Wsr.transpose(2, 3, 1, 0).reshape(12, 128, C).transpose(1, 0, 2)
    )
    # proj weights: row 64h + vcol(j) = Wp[:, 48h+j] -> [128, 4, C] in fp16
    wp_t = np.zeros((NH * 64, C), np.float32)
    for h in range(NH):
        wp_t[64 * h + _vcol(h), :] = Wp[:, HD * h : HD * (h + 1)].T
    wp_t = np.ascontiguousarray(
        wp_t.reshape(4, 128, C).transpose(1, 0, 2)
    ).astype(np.float16)
    # positional bias, pre-scaled, padded to 64-channel heads -> [128, HP, NK]
    pos_flat = (rel_h + rel_w).reshape(NH, HD, NK).astype(np.float32) * SCALE
    pos_t = np.zeros((NH * 64, NK), np.float32)
    for h in range(NH):
        pos_t[64 * h : 64 * h + HD, :] = pos_flat[h]
    pos_t = np.ascontiguousarray(pos_t.reshape(HP, 128, NK).transpose(1, 0, 2))
    bsr_t = np.ascontiguousarray(bsr.reshape(CT, 128).T)
    bp_t = np.ascontiguousarray(bp.reshape(CT, 128).T)

    in_maps = []
    for core in range(8):
        b, s = core // 2, core % 2
        xb = x[b].reshape(C, N)
        xf_t = np.ascontiguousarray(
            xb.reshape(CT, 128, 2, N // 2).transpose(1, 2, 0, 3)
        )
        pos_c = pos_t
        if s == 1:
            # own query-half first; key order swapped to match conv output
            xf_t = np.ascontiguousarray(xf_t[:, ::-1])
            pos_c = np.ascontiguousarray(
                np.concatenate([pos_t[:, :, 512:], pos_t[:, :, :512]], axis=2)
            )
        in_maps.append(
            {
                "xf": xf_t,
                "wq": wq_t,
                "wk": wk_t,
                "wv": wv_t,
                "wsr": wsr_t,
                "wp": wp_t,
                "pos": pos_c,
                "bsr": bsr_t,
                "bp": bp_t,
            }
        )
    return in_maps


def assemble_output(results):
    out = np.empty((B, C, N), np.float32)
    for core in range(8):
        b, s = core // 2, core % 2
        out[b, :, s * NQ : (s + 1) * NQ] = (
            results[core]["out"].transpose(1, 0, 2).reshape(C, NQ)
        )
    return out.reshape(B, C, H, W)


_cache = threading.Lock()
_program = None


def get_program():
    global _program
    with _cache:
        if _program is None:
            _program = build_program()
    return _program


def run(inputs, **kwargs):
    nc = get_program()
    in_maps = prep_inputs(inputs)
    res = run_bass_kernel_spmd(nc, in_maps, core_ids=list(range(8)), **kwargs)
    return assemble_output(res.results), res


def kernel(**inputs):
    out, _ = run(inputs)
    return out


# revision 18
# speedup vs baseline: 1.3638x; 1.3638x over previous
"""Trainium2 Bass kernel for PVT-style MHSA with spatial reduction.

Problem (hardcoded): B=4, C=384, H=W=64, NH=8 heads, HD=48, SR=2.
  q = Wq@x;  xsr = conv2x2s2(x, Wsr)+bsr;  k = (Wk@xsr + pos)*scale;  v = Wv@xsr
  attn = softmax(q^T k);  out = Wp@(v attn) + bp

Sharding: 8 cores = (batch b, query-half s).  Each core computes the full
conv/k/v for its batch (duplicated across the 2 cores of a batch) and
attention + projection for its 2048 queries.  No collectives.

Device notes (v6):
  - everything fp16 except PSUM accumulation (f32) and the f32 bias /
    rowsum path; fp16 matmuls measure ~480ns vs ~595ns for f32r, and
    input DMA halves.
  - attention computed transposed: attnT[m, n] = sum_d k[d, m] q[d, n]; the
    key-axis softmax reduction rides the AV matmul via an all-ones column in
    v^T at head-local col 32 (rowsums on 32-aligned PSUM partitions 32/96);
    QK^T pairs row-packed, AV pairs col-packed via tile_position.
  - exp split: ~31% of key-tiles on DVE via a Schraudolph bit-trick
    (x*K+B -> int16, bitcast fp16), the rest on ACT's spline exp.
  - softmax reciprocal: rowsum pair copied out of PSUM, scatter-DMA'd to
    [64,16], reciprocal there (16 el/lane not 512), DRAM bounce,
    partition-broadcast, normalize — software-pipelined two tile-groups
    deep so the DMA latency never blocks the engine queues.
  - phase A (conv/k/q/v) is interleaved INTO the attention group loop as
    front-of-group work units: the PE stays dense (no HAM re-throttle to
    1.2 GHz) and the first attention group starts ~70us earlier than a
    serial phase A.  Phase A shares the phase-B PSUM pools (prps/qkps).
"""

import threading

import numpy as np

import concourse.bass as bass
import concourse.mybir as mybir
import concourse.tile as tile
from concourse import bacc
from concourse.bass import ts
from concourse.bass_utils import run_bass_kernel_spmd

B, C, H, W = 4, 384, 64, 64
NH, HD, SR = 8, 48, 2
SCALE = HD ** -0.5
Hs, Ws = H // SR, W // SR
NK = Hs * Ws            # 1024 keys
N = H * W               # 4096 queries / batch
NQ = N // 2             # 2048 queries / core
CT = C // 128           # 3 c-tiles
HP = NH // 2            # 4 head-pair tiles
NB = NQ // 512          # 4 query blocks / core
MT = NK // 128          # 8 key tiles

F32 = mybir.dt.float32
F16 = mybir.dt.float16
I16 = mybir.dt.int16
AF = mybir.ActivationFunctionType
ALU = mybir.AluOpType

# fp16 Schraudolph: i16 = x*(1024/ln2) + (15*1024 - 1024*c - 0.5), bitcast f16
TRICK_K = 1024.0 / float(np.log(2.0))
TRICK_B = 15.0 * 1024.0 - 1024.0 * 0.043677448 - 0.5

DEFAULT_CFG = dict(
    qk_bufs=2, av_bufs=2, pr_bufs=2, e_bufs=2, r_bufs=3, dr_bufs=3,
    dve_mi=2,       # key-tiles per (nb,hp) exp'd on DVE (of MT=8)
    dve_extra=2,    # if >0: every dve_extra-th (nb,hp) group gets one more
)


def build_program(**cfg):
    cfg = {**DEFAULT_CFG, **cfg}
    nc = bacc.Bacc(None, target_bir_lowering=False)

    xf = nc.dram_tensor("xf", [128, 2, CT, N // 2], F16, kind="ExternalInput")
    wq = nc.dram_tensor("wq", [128, CT, 512], F16, kind="ExternalInput")
    wk = nc.dram_tensor("wk", [128, CT, 512], F16, kind="ExternalInput")
    wv = nc.dram_tensor("wv", [128, CT, 512], F16, kind="ExternalInput")
    wsr = nc.dram_tensor("wsr", [128, 12, C], F16, kind="ExternalInput")
    wp = nc.dram_tensor("wp", [128, 4, C], F16, kind="ExternalInput")
    pos = nc.dram_tensor("pos", [128, HP, NK], F16, kind="ExternalInput")
    bsr = nc.dram_tensor("bsr", [128, CT], F32, kind="ExternalInput")
    bp = nc.dram_tensor("bp", [128, CT], F32, kind="ExternalInput")
    out = nc.dram_tensor("out", [128, CT, NQ], F32, kind="ExternalOutput")

    with tile.TileContext(nc) as tc:
        with (
            tc.tile_pool(name="constp", bufs=1) as constp,
            tc.tile_pool(name="actp", bufs=1) as actp,
            tc.tile_pool(name="epool", bufs=cfg["e_bufs"]) as epool,
            tc.tile_pool(name="rpool", bufs=cfg["r_bufs"]) as rpool,
            tc.tile_pool(name="drp", bufs=cfg["dr_bufs"], space="DRAM") as drp,
            tc.tile_pool(name="qkps", bufs=cfg["qk_bufs"], space="PSUM") as qkps,
            tc.tile_pool(name="avps", bufs=cfg["av_bufs"], space="PSUM") as avps,
            tc.tile_pool(name="prps", bufs=cfg["pr_bufs"], space="PSUM") as prps,
        ):
            wq_sb = constp.tile([128, CT, 512], F16, name="wq_sb")
            wk_sb = constp.tile([128, CT, 512], F16, name="wk_sb")
            wv_sb = constp.tile([128, CT, 512], F16, name="wv_sb")
            wsr_sb = constp.tile([128, 12, C], F16, name="wsr_sb")
            wp_sb = constp.tile([128, 4, C], F16, name="wp_sb")
            pos_sb = constp.tile([128, HP, NK], F16, name="pos_sb")
            bsr_sb = constp.tile([128, CT], F32, name="bsr_sb")
            bp_sb = constp.tile([128, CT], F32, name="bp_sb")

            xf_sb = actp.tile([128, 2, CT, N // 2], F16, name="xf_sb")
            xsr_sb = actp.tile([128, CT, NK], F16, name="xsr_sb")
            q_sb = actp.tile([128, HP, NQ], F16, name="q_sb")
            k_sb = actp.tile([128, HP, NK], F16, name="k_sb")
            vt_sb = actp.tile([128, MT, 512], F16, name="vt_sb")
            o_sb = actp.tile([128, HP, NQ], F16, name="o_sb")
            outp_sb = actp.tile([128, CT, NQ], F32, name="outp_sb")

            # ACT HWDGE ring: weights/bias/pos (ACT is idle until exps)
            nc.scalar.dma_start(wsr_sb[:], wsr[:])
            nc.scalar.dma_start(wk_sb[:], wk[:])
            nc.scalar.dma_start(wq_sb[:], wq[:])
            nc.scalar.dma_start(bsr_sb[:], bsr[:])
            nc.scalar.dma_start(pos_sb[:], pos[:])
            nc.scalar.dma_start(wv_sb[:], wv[:])
            nc.scalar.dma_start(wp_sb[:], wp[:])
            nc.scalar.dma_start(bp_sb[:], bp[:])
            # SP HWDGE ring: activations, ordered by first use
            nc.sync.dma_start(xf_sb[:, 0], xf[:, 0])
            nc.sync.dma_start(xf_sb[:, 1], xf[:, 1])

            # ---- phase A work units (emitted interleaved with groups) ---
            def emit_conv(mb, ot):
                p = prps.tile([128, 512], F32, name="pp", tag="pp")
                n_mm = 0
                for didj in range(4):
                    di, dj = didj // 2, didj % 2
                    for ci in range(CT):
                        base = xf_sb[:]
                        rhs = bass.AP(
                            tensor=base.tensor,
                            offset=base.offset
                            + mb * (CT * N // 2)
                            + ci * (N // 2)
                            + di * W
                            + dj,
                            ap=[base.ap[0], [2 * W, Hs // 2], [2, Ws]],
                        )
                        nc.tensor.matmul(
                            p[:],
                            wsr_sb[:, didj * CT + ci, ts(ot, 128)],
                            rhs,
                            start=(n_mm == 0),
                            stop=(n_mm == 11),
                        )
                        n_mm += 1
                nc.vector.tensor_scalar_add(
                    xsr_sb[:, ot, ts(mb, 512)], p[:], bsr_sb[:, ot : ot + 1]
                )

            def emit_k(hp, mb):
                p = prps.tile([128, 512], F32, name="pp", tag="pp")
                for ci in range(CT):
                    nc.tensor.matmul(
                        p[:],
                        wk_sb[:, ci, ts(hp, 128)],
                        xsr_sb[:, ci, ts(mb, 512)],
                        start=(ci == 0),
                        stop=(ci == CT - 1),
                    )
                nc.vector.tensor_add(
                    k_sb[:, hp, ts(mb, 512)], p[:], pos_sb[:, hp, ts(mb, 512)]
                )

            def emit_q(ot, nb):
                p = prps.tile([128, 512], F32, name="pp", tag="pp")
                for ci in range(CT):
                    nc.tensor.matmul(
                        p[:],
                        wq_sb[:, ci, ts(ot, 128)],
                        xf_sb[:, 0, ci, ts(nb, 512)],
                        start=(ci == 0),
                        stop=(ci == CT - 1),
                    )
                nc.vector.tensor_copy(q_sb[:, ot, ts(nb, 512)], p[:])

            def emit_vt(mi):
                p = prps.tile([128, 512], F32, name="pp", tag="pp")
                for ci in range(CT):
                    nc.tensor.matmul(
                        p[:],
                        xsr_sb[:, ci, ts(mi, 128)],
                        wv_sb[:, ci, :],
                        start=(ci == 0),
                        stop=(ci == CT - 1),
                    )
                nc.vector.tensor_copy(vt_sb[:, mi, :], p[:])
                base = vt_sb[:]
                ones_ap = bass.AP(
                    tensor=base.tensor,
                    offset=base.offset + mi * 512 + 32,
                    ap=[base.ap[0], [64, NH]],
                )
                nc.gpsimd.memset(ones_ap, 1.0)

            # ---- normalize chain (2-group software pipeline) -------------
            def emit_proj(nb):
                for ot in range(CT):
                    p = prps.tile([128, 512], F32, name="pp", tag="pp")
                    for dd in range(4):
                        nc.tensor.matmul(
                            p[:],
                            wp_sb[:, dd, ts(ot, 128)],
                            o_sb[:, dd, ts(nb, 512)],
                            start=(dd == 0),
                            stop=(dd == 3),
                        )
                    nc.vector.tensor_scalar_add(
                        outp_sb[:, ot, ts(nb, 512)],
                        p[:],
                        bp_sb[:, ot : ot + 1],
                    )
                nc.sync.dma_start(
                    out[:, :, ts(nb, 512)], outp_sb[:, :, ts(nb, 512)]
                )

            def emit_norm(s):
                nc.vector.tensor_mul(
                    o_sb[:, s["hp"], ts(s["nb"], 512)], s["oav"][:], s["rb"][:]
                )
                if s["hp"] == HP - 1:
                    emit_proj(s["nb"])

            def emit_extract(s):
                ex = rpool.tile([33, 512], F32, name="ex", tag="ex")
                nc.vector.tensor_copy(ex[0:1, :], s["oav"][32:33, :])
                nc.vector.tensor_copy(ex[32:33, :], s["oav"][96:97, :])
                rs = rpool.tile([64, 16], F32, name="rs", tag="rs")
                nc.gpsimd.dma_start(rs[0:32, :], ex[0:1, :])
                nc.gpsimd.dma_start(rs[32:64, :], ex[32:33, :])
                s["rs"] = rs

            def emit_recip_bcast(s):
                rr = rpool.tile([64, 16], F32, name="rr", tag="rr")
                nc.vector.reciprocal(out=rr[:], in_=s["rs"][:])
                r2d = drp.tile([2, 512], F32, name="r2d", tag="r2d")
                nc.gpsimd.dma_start(r2d[0:1], rr[0:32, :])
                nc.gpsimd.dma_start(r2d[1:2], rr[32:64, :])
                rb = rpool.tile([128, 512], F32, name="rb", tag="rb")
                nc.sync.dma_start(
                    rb[0:64, :].unsqueeze(1),
                    r2d[0:1, :].partition_broadcast(64),
                )
                nc.sync.dma_start(
                    rb[64:128, :].unsqueeze(1),
                    r2d[1:2, :].partition_broadcast(64),
                )
                s["rb"] = rb

            # ---- head: minimum phase A to unblock group (nb0, hp0) ------
            for ot in range(CT):
                emit_conv(0, ot)
            emit_k(0, 0)
            emit_q(0, 0)
            for mi in range(2):
                emit_vt(mi)

            groups = [(nb, hp) for nb in range(NB) for hp in range(HP)]

            # group 0 mi-loop insertions: rest of phase A that feeds the
            # other nb0 groups (conv mb1, all k, all vt)
            g0_mid = {
                0: lambda: (emit_conv(1, 0), emit_k(1, 0)),
                1: lambda: (emit_conv(1, 1), emit_k(2, 0)),
                2: lambda: (emit_conv(1, 2), emit_k(3, 0), emit_vt(2)),
                3: lambda: (emit_k(0, 1), emit_vt(3)),
                4: lambda: (emit_k(1, 1), emit_vt(4)),
                5: lambda: (emit_k(2, 1), emit_vt(5)),
                6: lambda: (emit_k(3, 1), emit_vt(6)),
                7: lambda: emit_vt(7),
            }

            def front_work(g):
                # emit the next group's q one group ahead of use
                if g + 1 < len(groups):
                    nb_n, hp_n = groups[g + 1]
                    if not (nb_n == 0 and hp_n == 0):
                        emit_q(hp_n, nb_n)

            # DVE-trick exp tiles per group: none in group 0 (DVE busy with
            # phase A drains), target total = 31% of 128 tiles
            dve_total = 16 * cfg["dve_mi"] + (
                (16 // cfg["dve_extra"]) if cfg["dve_extra"] else 0
            )
            dve_sched = [0] * 16
            rem = dve_total
            gi = 1
            while rem > 0:
                if dve_sched[gi] < 3:
                    dve_sched[gi] += 1
                    rem -= 1
                gi = gi + 1 if gi < 15 else 1

            state = {}
            for g, (nb, hp) in enumerate(groups):
                if g >= 2:
                    emit_norm(state.pop(g - 2))
                front_work(g)
                if g >= 1:
                    emit_extract(state[g - 1])

                dve_n = dve_sched[g]
                e_sb = epool.tile([128, MT, 1024], F16, name="e", tag="e")
                for mi in range(MT):
                    qk = qkps.tile([128, 1024], F32, name="qk", tag="qk")
                    nc.tensor.matmul(
                        qk[:, 0:512],
                        k_sb[0:64, hp, ts(mi, 128)],
                        q_sb[0:64, hp, ts(nb, 512)],
                        start=True,
                        stop=True,
                        tile_position=(0, 0),
                    )
                    nc.tensor.matmul(
                        qk[:, 512:1024],
                        k_sb[64:128, hp, ts(mi, 128)],
                        q_sb[64:128, hp, ts(nb, 512)],
                        start=True,
                        stop=True,
                        tile_position=(64, 0),
                    )
                    if mi >= MT - dve_n:
                        nc.vector.tensor_scalar(
                            out=e_sb[:, mi, :].bitcast(I16),
                            in0=qk[:],
                            scalar1=TRICK_K,
                            scalar2=TRICK_B,
                            op0=ALU.mult,
                            op1=ALU.add,
                        )
                    else:
                        nc.scalar.activation(
                            out=e_sb[:, mi, :], in_=qk[:], func=AF.Exp
                        )
                    if g == 0 and mi in g0_mid:
                        g0_mid[mi]()

                if g >= 1:
                    emit_recip_bcast(state[g - 1])

                oav = avps.tile([128, 512], F32, name="oav", tag="oav")
                for mi in range(MT):
                    nc.tensor.matmul(
                        oav[0:64, :],
                        vt_sb[:, mi, 128 * hp : 128 * hp + 64],
                        e_sb[:, mi, 0:512],
                        start=(mi == 0),
                        stop=(mi == MT - 1),
                        tile_position=(0, 0),
                        skip_group_check=True,
                    )
                    nc.tensor.matmul(
                        oav[64:128, :],
                        vt_sb[:, mi, 128 * hp + 64 : 128 * (hp + 1)],
                        e_sb[:, mi, 512:1024],
                        start=(mi == 0),
                        stop=(mi == MT - 1),
                        tile_position=(0, 64),
                        skip_group_check=True,
                    )
                state[g] = {"nb": nb, "hp": hp, "oav": oav}

            # drain the pipeline
            ng = len(groups)
            emit_norm(state.pop(ng - 2))
            emit_extract(state[ng - 1])
            emit_recip_bcast(state[ng - 1])
            emit_norm(state.pop(ng - 1))

    nc.compile()
    return nc


def _pad_cols(w):
    """[C, C] weight -> [C, 512]: col 64h+j = w[48h+j, :] (j < 48)."""
    wt = np.zeros((C, NH * 64), np.float32)
    for h in range(NH):
        wt[:, 64 * h : 64 * h + HD] = w[HD * h : HD * (h + 1), :].T
    return wt


# v/proj head-local channel placement: ones column at local col 32 so the
# rowsum lands on a 32-aligned PSUM partition; channel d -> col d (d<32)
# else d+1
_VCOL = np.array([d if d < 32 else d + 1 for d in range(HD)])


def _ctile(w, dt=np.float16):
    """[C, F] -> [128, CT, F] (partition-major c-tiles)."""
    return np.ascontiguousarray(
        w.reshape(CT, 128, -1).transpose(1, 0, 2)
    ).astype(dt)


def prep_inputs(inputs):
    x = np.asarray(inputs["x"], np.float32)
    Wq = np.asarray(inputs["Wq"], np.float32)
    Wk = np.asarray(inputs["Wk"], np.float32)
    Wv = np.asarray(inputs["Wv"], np.float32)
    Wsr = np.asarray(inputs["Wsr"], np.float32)
    bsr = np.asarray(inputs["bsr"], np.float32)
    Wp = np.asarray(inputs["Wp"], np.float32)
    bp = np.asarray(inputs["bp"], np.float32)
    rel_h = np.asarray(inputs["rel_h"], np.float32)
    rel_w = np.asarray(inputs["rel_w"], np.float32)

    wq_t = _ctile(_pad_cols(Wq))
    wk_t = _ctile(_pad_cols(Wk) * SCALE)
    wv_pad = np.zeros((C, NH * 64), np.float32)
    for h in range(NH):
        wv_pad[:, 64 * h + _VCOL] = Wv[HD * h : HD * (h + 1), :].T
    wv_t = _ctile(wv_pad)
    # conv weights: rows ordered (di, dj, c) -> [128, 12, C] (didj, ci) tiles
    wsr_t = np.ascontiguousarray(
        Wsr.transpose(2, 3, 1, 0).reshape(12, 128, C).transpose(1, 0, 2)
    ).astype(np.float16)
    # proj weights: row 64h + vcol(j) = Wp[:, 48h+j] -> [128, 4, C] in fp16
    wp_t = np.zeros((NH * 64, C), np.float32)
    for h in range(NH):
        wp_t[64 * h + _VCOL, :] = Wp[:, HD * h : HD * (h + 1)].T
    wp_t = np.ascontiguousarray(
        wp_t.reshape(4, 128, C).transpose(1, 0, 2)
    ).astype(np.float16)
    # positional bias, pre-scaled, padded to 64-channel heads -> [128, HP, NK]
    pos_flat = (rel_h + rel_w).reshape(NH, HD, NK).astype(np.float32) * SCALE
    pos_t = np.zeros((NH * 64, NK), np.float32)
    for h in range(NH):
        pos_t[64 * h : 64 * h + HD, :] = pos_flat[h]
    pos_t = np.ascontiguousarray(
        pos_t.reshape(HP, 128, NK).transpose(1, 0, 2)
    ).astype(np.float16)
    bsr_t = np.ascontiguousarray(bsr.reshape(CT, 128).T)
    bp_t = np.ascontiguousarray(bp.reshape(CT, 128).T)

    in_maps = []
    for core in range(8):
        b, s = core // 2, core % 2
        xb = x[b].reshape(C, N)
        xf_t = np.ascontiguousarray(
            xb.reshape(CT, 128, 2, N // 2).transpose(1, 2, 0, 3)
        ).astype(np.float16)
        pos_c = pos_t
        if s == 1:
            # own query-half first; key order swapped to match conv output
            xf_t = np.ascontiguousarray(xf_t[:, ::-1])
            pos_c = np.ascontiguousarray(
                np.concatenate([pos_t[:, :, 512:], pos_t[:, :, :512]], axis=2)
            )
        in_maps.append(
            {
                "xf": xf_t,
                "wq": wq_t,
                "wk": wk_t,
                "wv": wv_t,
                "wsr": wsr_t,
                "wp": wp_t,
                "pos": pos_c,
                "bsr": bsr_t,
                "bp": bp_t,
            }
        )
    return in_maps


def assemble_output(results):
    out = np.empty((B, C, N), np.float32)
    for core in range(8):
        b, s = core // 2, core % 2
        out[b, :, s * NQ : (s + 1) * NQ] = (
            results[core]["out"].transpose(1, 0, 2).reshape(C, NQ)
        )
    return out.reshape(B, C, H, W)


_cache = threading.Lock()
_program = None


def get_program():
    global _program
    with _cache:
        if _program is None:
            _program = build_program()
    return _program


def run(inputs, **kwargs):
    nc = get_program()
    in_maps = prep_inputs(inputs)
    res = run_bass_kernel_spmd(nc, in_maps, core_ids=list(range(8)), **kwargs)
    return assemble_output(res.results), res


def kernel(**inputs):
    out, _ = run(inputs)
    return out


# revision 21
# speedup vs baseline: 1.3928x; 1.0213x over previous
"""Trainium2 Bass kernel for PVT-style MHSA with spatial reduction.

Problem (hardcoded): B=4, C=384, H=W=64, NH=8 heads, HD=48, SR=2.
  q = Wq@x;  xsr = conv2x2s2(x, Wsr)+bsr;  k = (Wk@xsr + pos)*scale;  v = Wv@xsr
  attn = softmax(q^T k);  out = Wp@(v attn) + bp

Sharding: 8 cores = (batch b, query-half s).  Each core computes the full
conv/k/v for its batch (duplicated across the 2 cores of a batch) and
attention + projection for its 2048 queries.  No collectives.

Device notes (v6):
  - everything fp16 except PSUM accumulation (f32) and the f32 bias /
    rowsum path; fp16 matmuls measure ~480ns vs ~595ns for f32r, and
    input DMA halves.
  - attention computed transposed: attnT[m, n] = sum_d k[d, m] q[d, n]; the
    key-axis softmax reduction rides the AV matmul via an all-ones column in
    v^T at head-local col 32 (rowsums on 32-aligned PSUM partitions 32/96);
    QK^T pairs row-packed, AV pairs col-packed via tile_position.
  - exp split: ~31% of key-tiles on DVE via a Schraudolph bit-trick
    (x*K+B -> int16, bitcast fp16), the rest on ACT's spline exp.
  - softmax reciprocal: rowsum pair copied out of PSUM, scatter-DMA'd to
    [64,16], reciprocal there (16 el/lane not 512), DRAM bounce,
    partition-broadcast, normalize — software-pipelined two tile-groups
    deep so the DMA latency never blocks the engine queues.
  - phase A (conv/k/q/v) is interleaved INTO the attention group loop as
    front-of-group work units: the PE stays dense (no HAM re-throttle to
    1.2 GHz) and the first attention group starts ~70us earlier than a
    serial phase A.  Phase A shares the phase-B PSUM pools (prps/qkps).
"""

import threading

import numpy as np

import concourse.bass as bass
import concourse.mybir as mybir
import concourse.tile as tile
from concourse import bacc
from concourse.bass import ts
from concourse.bass_utils import run_bass_kernel_spmd

B, C, H, W = 4, 384, 64, 64
NH, HD, SR = 8, 48, 2
SCALE = HD ** -0.5
Hs, Ws = H // SR, W // SR
NK = Hs * Ws            # 1024 keys
N = H * W               # 4096 queries / batch
NQ = N // 2             # 2048 queries / core
CT = C // 128           # 3 c-tiles
HP = NH // 2            # 4 head-pair tiles
NB = NQ // 512          # 4 query blocks / core
MT = NK // 128          # 8 key tiles

F32 = mybir.dt.float32
F16 = mybir.dt.float16
I16 = mybir.dt.int16
AF = mybir.ActivationFunctionType
ALU = mybir.AluOpType

# fp16 Schraudolph: i16 = x*(1024/ln2) + (15*1024 - 1024*c - 0.5), bitcast f16
TRICK_K = 1024.0 / float(np.log(2.0))
TRICK_B = 15.0 * 1024.0 - 1024.0 * 0.043677448 - 0.5

DEFAULT_CFG = dict(
    qk_bufs=2, av_bufs=2, pr_bufs=2, e_bufs=2, r_bufs=3, dr_bufs=3,
    dve_mi=2,       # key-tiles per (nb,hp) exp'd on DVE (of MT=8)
    dve_extra=2,    # if >0: every dve_extra-th (nb,hp) group gets one more
)


def build_program(**cfg):
    cfg = {**DEFAULT_CFG, **cfg}
    nc = bacc.Bacc(None, target_bir_lowering=False)

    xf = nc.dram_tensor("xf", [128, 2, CT, N // 2], F16, kind="ExternalInput")
    wq = nc.dram_tensor("wq", [128, CT, 512], F16, kind="ExternalInput")
    wk = nc.dram_tensor("wk", [128, CT, 512], F16, kind="ExternalInput")
    wv = nc.dram_tensor("wv", [128, CT, 512], F16, kind="ExternalInput")
    wsr = nc.dram_tensor("wsr", [128, 12, C], F16, kind="ExternalInput")
    wp = nc.dram_tensor("wp", [128, 4, C], F16, kind="ExternalInput")
    pos = nc.dram_tensor("pos", [128, HP, NK], F16, kind="ExternalInput")
    bsr = nc.dram_tensor("bsr", [128, CT], F32, kind="ExternalInput")
    bp = nc.dram_tensor("bp", [128, CT], F32, kind="ExternalInput")
    out = nc.dram_tensor("out", [128, CT, NQ], F32, kind="ExternalOutput")

    with tile.TileContext(nc) as tc:
        with (
            tc.tile_pool(name="constp", bufs=1) as constp,
            tc.tile_pool(name="actp", bufs=1) as actp,
            tc.tile_pool(name="epool", bufs=cfg["e_bufs"]) as epool,
            tc.tile_pool(name="rpool", bufs=cfg["r_bufs"]) as rpool,
            tc.tile_pool(name="drp", bufs=cfg["dr_bufs"], space="DRAM") as drp,
            tc.tile_pool(name="qkps", bufs=cfg["qk_bufs"], space="PSUM") as qkps,
            tc.tile_pool(name="avps", bufs=cfg["av_bufs"], space="PSUM") as avps,
            tc.tile_pool(name="prps", bufs=cfg["pr_bufs"], space="PSUM") as prps,
        ):
            wq_sb = constp.tile([128, CT, 512], F16, name="wq_sb")
            wk_sb = constp.tile([128, CT, 512], F16, name="wk_sb")
            wv_sb = constp.tile([128, CT, 512], F16, name="wv_sb")
            wsr_sb = constp.tile([128, 12, C], F16, name="wsr_sb")
            wp_sb = constp.tile([128, 4, C], F16, name="wp_sb")
            pos_sb = constp.tile([128, HP, NK], F16, name="pos_sb")
            bsr_sb = constp.tile([128, CT], F32, name="bsr_sb")
            bp_sb = constp.tile([128, CT], F32, name="bp_sb")

            xf_sb = actp.tile([128, 2, CT, N // 2], F16, name="xf_sb")
            xsr_sb = actp.tile([128, CT, NK], F16, name="xsr_sb")
            q_sb = actp.tile([128, HP, NQ], F16, name="q_sb")
            k_sb = actp.tile([128, HP, NK], F16, name="k_sb")
            vt_sb = actp.tile([128, MT, 512], F16, name="vt_sb")
            o_sb = actp.tile([128, HP, NQ], F16, name="o_sb")
            outp_sb = actp.tile([128, CT, NQ], F32, name="outp_sb")

            # ACT HWDGE ring: weights/bias/pos (ACT is idle until exps)
            nc.scalar.dma_start(wsr_sb[:], wsr[:])
            nc.scalar.dma_start(wk_sb[:], wk[:])
            nc.scalar.dma_start(wq_sb[:], wq[:])
            nc.scalar.dma_start(bsr_sb[:], bsr[:])
            nc.scalar.dma_start(pos_sb[:], pos[:])
            nc.scalar.dma_start(wv_sb[:], wv[:])
            nc.scalar.dma_start(wp_sb[:], wp[:])
            nc.scalar.dma_start(bp_sb[:], bp[:])
            # SP HWDGE ring: activations, ordered by first use
            nc.sync.dma_start(xf_sb[:, 0], xf[:, 0])
            nc.sync.dma_start(xf_sb[:, 1], xf[:, 1])

            # ---- phase A work units (emitted interleaved with groups) ---
            def emit_conv(mb, ot):
                p = prps.tile([128, 512], F32, name="pp", tag="pp")
                n_mm = 0
                for didj in range(4):
                    di, dj = didj // 2, didj % 2
                    for ci in range(CT):
                        base = xf_sb[:]
                        rhs = bass.AP(
                            tensor=base.tensor,
                            offset=base.offset
                            + mb * (CT * N // 2)
                            + ci * (N // 2)
                            + di * W
                            + dj,
                            ap=[base.ap[0], [2 * W, Hs // 2], [2, Ws]],
                        )
                        nc.tensor.matmul(
                            p[:],
                            wsr_sb[:, didj * CT + ci, ts(ot, 128)],
                            rhs,
                            start=(n_mm == 0),
                            stop=(n_mm == 11),
                        )
                        n_mm += 1
                nc.vector.tensor_scalar_add(
                    xsr_sb[:, ot, ts(mb, 512)], p[:], bsr_sb[:, ot : ot + 1]
                )

            def emit_k(hp, mb):
                p = prps.tile([128, 512], F32, name="pp", tag="pp")
                for ci in range(CT):
                    nc.tensor.matmul(
                        p[:],
                        wk_sb[:, ci, ts(hp, 128)],
                        xsr_sb[:, ci, ts(mb, 512)],
                        start=(ci == 0),
                        stop=(ci == CT - 1),
                    )
                nc.vector.tensor_add(
                    k_sb[:, hp, ts(mb, 512)], p[:], pos_sb[:, hp, ts(mb, 512)]
                )

            def emit_q(ot, nb):
                p = prps.tile([128, 512], F32, name="pp", tag="pp")
                for ci in range(CT):
                    nc.tensor.matmul(
                        p[:],
                        wq_sb[:, ci, ts(ot, 128)],
                        xf_sb[:, 0, ci, ts(nb, 512)],
                        start=(ci == 0),
                        stop=(ci == CT - 1),
                    )
                nc.vector.tensor_copy(q_sb[:, ot, ts(nb, 512)], p[:])

            def emit_vt(mi):
                p = prps.tile([128, 512], F32, name="pp", tag="pp")
                for ci in range(CT):
                    nc.tensor.matmul(
                        p[:],
                        xsr_sb[:, ci, ts(mi, 128)],
                        wv_sb[:, ci, :],
                        start=(ci == 0),
                        stop=(ci == CT - 1),
                    )
                nc.vector.tensor_copy(vt_sb[:, mi, :], p[:])
                base = vt_sb[:]
                ones_ap = bass.AP(
                    tensor=base.tensor,
                    offset=base.offset + mi * 512 + 32,
                    ap=[base.ap[0], [64, NH]],
                )
                nc.gpsimd.memset(ones_ap, 1.0)

            # ---- normalize chain (2-group software pipeline) -------------
            def emit_proj(nb):
                for ot in range(CT):
                    p = prps.tile([128, 512], F32, name="pp", tag="pp")
                    for dd in range(4):
                        nc.tensor.matmul(
                            p[:],
                            wp_sb[:, dd, ts(ot, 128)],
                            o_sb[:, dd, ts(nb, 512)],
                            start=(dd == 0),
                            stop=(dd == 3),
                        )
                    nc.vector.tensor_scalar_add(
                        outp_sb[:, ot, ts(nb, 512)],
                        p[:],
                        bp_sb[:, ot : ot + 1],
                    )
                nc.sync.dma_start(
                    out[:, :, ts(nb, 512)], outp_sb[:, :, ts(nb, 512)]
                )

            def emit_norm(s):
                nc.vector.tensor_mul(
                    o_sb[:, s["hp"], ts(s["nb"], 512)], s["oav"][:], s["rb"][:]
                )
                if s["hp"] == HP - 1:
                    emit_proj(s["nb"])

            def emit_extract(s):
                ex = rpool.tile([33, 512], F32, name="ex", tag="ex")
                nc.vector.tensor_copy(ex[0:1, :], s["oav"][32:33, :])
                nc.vector.tensor_copy(ex[32:33, :], s["oav"][96:97, :])
                rs = rpool.tile([64, 16], F32, name="rs", tag="rs")
                nc.gpsimd.dma_start(rs[0:32, :], ex[0:1, :])
                nc.gpsimd.dma_start(rs[32:64, :], ex[32:33, :])
                s["rs"] = rs

            def emit_recip_bcast(s):
                rr = rpool.tile([64, 16], F32, name="rr", tag="rr")
                nc.vector.reciprocal(out=rr[:], in_=s["rs"][:])
                r2d = drp.tile([2, 512], F32, name="r2d", tag="r2d")
                nc.gpsimd.dma_start(r2d[0:1], rr[0:32, :])
                nc.gpsimd.dma_start(r2d[1:2], rr[32:64, :])
                rb = rpool.tile([128, 512], F32, name="rb", tag="rb")
                nc.sync.dma_start(
                    rb[0:64, :].unsqueeze(1),
                    r2d[0:1, :].partition_broadcast(64),
                )
                nc.sync.dma_start(
                    rb[64:128, :].unsqueeze(1),
                    r2d[1:2, :].partition_broadcast(64),
                )
                s["rb"] = rb

            # ---- head: minimum phase A to unblock group (nb0, hp0) ------
            for ot in range(CT):
                emit_conv(0, ot)
            emit_k(0, 0)
            emit_q(0, 0)
            for mi in range(2):
                emit_vt(mi)

            groups = [(nb, hp) for nb in range(NB) for hp in range(HP)]

            # group 0 mi-loop insertions: rest of phase A that feeds the
            # other nb0 groups (conv mb1, all k, all vt)
            g0_mid = {
                0: lambda: (emit_conv(1, 0), emit_k(1, 0)),
                1: lambda: (emit_conv(1, 1), emit_k(2, 0)),
                2: lambda: (emit_conv(1, 2), emit_k(3, 0), emit_vt(2)),
                3: lambda: (emit_k(0, 1), emit_vt(3)),
                4: lambda: (emit_k(1, 1), emit_vt(4)),
                5: lambda: (emit_k(2, 1), emit_vt(5)),
                6: lambda: (emit_k(3, 1), emit_vt(6)),
                7: lambda: emit_vt(7),
            }

            def front_work(g):
                # emit the next group's q one group ahead of use
                if g + 1 < len(groups):
                    nb_n, hp_n = groups[g + 1]
                    if not (nb_n == 0 and hp_n == 0):
                        emit_q(hp_n, nb_n)

            # DVE-trick exp tiles per group: none in group 0 (DVE busy with
            # phase A drains), target total = 31% of 128 tiles
            dve_total = 16 * cfg["dve_mi"] + (
                (16 // cfg["dve_extra"]) if cfg["dve_extra"] else 0
            )
            dve_sched = [0] * 16
            rem = dve_total
            gi = 1
            while rem > 0:
                if dve_sched[gi] < 3:
                    dve_sched[gi] += 1
                    rem -= 1
                gi = gi + 1 if gi < 15 else 1

            def emit_warm():
                # cheap dependency-free matmul (32-row output to minimize
                # PSUM write traffic): keeps the HAM activity monitor from
                # re-throttling the PE clock during sub-us pipeline gaps
                p = prps.tile([128, 512], F32, name="pp", tag="pp")
                nc.tensor.matmul(
                    p[0:32, :], wp_sb[:, 0, 0:32], vt_sb[:, 0, :],
                    start=True, stop=True,
                )

            state = {}
            for g, (nb, hp) in enumerate(groups):
                if g >= 2:
                    emit_norm(state.pop(g - 2))
                front_work(g)
                if g >= 1:
                    emit_extract(state[g - 1])

                dve_n = dve_sched[g]
                e_sb = epool.tile([128, MT, 1024], F16, name="e", tag="e")
                for mi in range(MT):
                    qk = qkps.tile([128, 1024], F32, name="qk", tag="qk")
                    nc.tensor.matmul(
                        qk[:, 0:512],
                        k_sb[0:64, hp, ts(mi, 128)],
                        q_sb[0:64, hp, ts(nb, 512)],
                        start=True,
                        stop=True,
                        tile_position=(0, 0),
                    )
                    nc.tensor.matmul(
                        qk[:, 512:1024],
                        k_sb[64:128, hp, ts(mi, 128)],
                        q_sb[64:128, hp, ts(nb, 512)],
                        start=True,
                        stop=True,
                        tile_position=(64, 0),
                    )
                    if mi >= MT - dve_n:
                        nc.vector.tensor_scalar(
                            out=e_sb[:, mi, :].bitcast(I16),
                            in0=qk[:],
                            scalar1=TRICK_K,
                            scalar2=TRICK_B,
                            op0=ALU.mult,
                            op1=ALU.add,
                        )
                    else:
                        nc.scalar.activation(
                            out=e_sb[:, mi, :], in_=qk[:], func=AF.Exp
                        )
                    if g == 0 and mi in g0_mid:
                        g0_mid[mi]()
                    elif g >= 1 and mi in (2, 5):
                        emit_warm()

                if g >= 1:
                    emit_recip_bcast(state[g - 1])

                oav = avps.tile([128, 512], F32, name="oav", tag="oav")
                for mi in range(MT):
                    nc.tensor.matmul(
                        oav[0:64, :],
                        vt_sb[:, mi, 128 * hp : 128 * hp + 64],
                        e_sb[:, mi, 0:512],
                        start=(mi == 0),
                        stop=(mi == MT - 1),
                        tile_position=(0, 0),
                        skip_group_check=True,
                    )
                    nc.tensor.matmul(
                        oav[64:128, :],
                        vt_sb[:, mi, 128 * hp + 64 : 128 * (hp + 1)],
                        e_sb[:, mi, 512:1024],
                        start=(mi == 0),
                        stop=(mi == MT - 1),
                        tile_position=(0, 64),
                        skip_group_check=True,
                    )
                state[g] = {"nb": nb, "hp": hp, "oav": oav}
                if g == len(groups) - 1:
                    # start the last chain immediately to shorten the drain
                    emit_extract(state[g])
                    emit_recip_bcast(state[g])

            # drain the pipeline
            ng = len(groups)
            emit_norm(state.pop(ng - 2))
            emit_norm(state.pop(ng - 1))

    nc.compile()
    return nc


def _pad_cols(w):
    """[C, C] weight -> [C, 512]: col 64h+j = w[48h+j, :] (j < 48)."""
    wt = np.zeros((C, NH * 64), np.float32)
    for h in range(NH):
        wt[:, 64 * h : 64 * h + HD] = w[HD * h : HD * (h + 1), :].T
    return wt


# v/proj head-local channel placement: ones column at local col 32 so the
# rowsum lands on a 32-aligned PSUM partition; channel d -> col d (d<32)
# else d+1
_VCOL = np.array([d if d < 32 else d + 1 for d in range(HD)])


def _ctile(w, dt=np.float16):
    """[C, F] -> [128, CT, F] (partition-major c-tiles)."""
    return np.ascontiguousarray(
        w.reshape(CT, 128, -1).transpose(1, 0, 2)
    ).astype(dt)


def prep_inputs(inputs):
    x = np.asarray(inputs["x"], np.float32)
    Wq = np.asarray(inputs["Wq"], np.float32)
    Wk = np.asarray(inputs["Wk"], np.float32)
    Wv = np.asarray(inputs["Wv"], np.float32)
    Wsr = np.asarray(inputs["Wsr"], np.float32)
    bsr = np.asarray(inputs["bsr"], np.float32)
    Wp = np.asarray(inputs["Wp"], np.float32)
    bp = np.asarray(inputs["bp"], np.float32)
    rel_h = np.asarray(inputs["rel_h"], np.float32)
    rel_w = np.asarray(inputs["rel_w"], np.float32)

    wq_t = _ctile(_pad_cols(Wq))
    wk_t = _ctile(_pad_cols(Wk) * SCALE)
    wv_pad = np.zeros((C, NH * 64), np.float32)
    for h in range(NH):
        wv_pad[:, 64 * h + _VCOL] = Wv[HD * h : HD * (h + 1), :].T
    wv_t = _ctile(wv_pad)
    # conv weights: rows ordered (di, dj, c) -> [128, 12, C] (didj, ci) tiles
    wsr_t = np.ascontiguousarray(
        Wsr.transpose(2, 3, 1, 0).reshape(12, 128, C).transpose(1, 0, 2)
    ).astype(np.float16)
    # proj weights: row 64h + vcol(j) = Wp[:, 48h+j] -> [128, 4, C] in fp16
    wp_t = np.zeros((NH * 64, C), np.float32)
    for h in range(NH):
        wp_t[64 * h + _VCOL, :] = Wp[:, HD * h : HD * (h + 1)].T
    wp_t = np.ascontiguousarray(
        wp_t.reshape(4, 128, C).transpose(1, 0, 2)
    ).astype(np.float16)
    # positional bias, pre-scaled, padded to 64-channel heads -> [128, HP, NK]
    pos_flat = (rel_h + rel_w).reshape(NH, HD, NK).astype(np.float32) * SCALE
    pos_t = np.zeros((NH * 64, NK), np.float32)
    for h in range(NH):
        pos_t[64 * h : 64 * h + HD, :] = pos_flat[h]
    pos_t = np.ascontiguousarray(
        pos_t.reshape(HP, 128, NK).transpose(1, 0, 2)
    ).astype(np.float16)
    bsr_t = np.ascontiguousarray(bsr.reshape(CT, 128).T)
    bp_t = np.ascontiguousarray(bp.reshape(CT, 128).T)

    in_maps = []
    for core in range(8):
        b, s = core // 2, core % 2
        xb = x[b].reshape(C, N)
        xf_t = np.ascontiguousarray(
            xb.reshape(CT, 128, 2, N // 2).transpose(1, 2, 0, 3)
        ).astype(np.float16)
        pos_c = pos_t
        if s == 1:
            # own query-half first; key order swapped to match conv output
            xf_t = np.ascontiguousarray(xf_t[:, ::-1])
            pos_c = np.ascontiguousarray(
                np.concatenate([pos_t[:, :, 512:], pos_t[:, :, :512]], axis=2)
            )
        in_maps.append(
            {
                "xf": xf_t,
                "wq": wq_t,
                "wk": wk_t,
                "wv": wv_t,
                "wsr": wsr_t,
                "wp": wp_t,
                "pos": pos_c,
                "bsr": bsr_t,
                "bp": bp_t,
            }
        )
    return in_maps


def assemble_output(results):
    out = np.empty((B, C, N), np.float32)
    for core in range(8):
        b, s = core // 2, core % 2
        out[b, :, s * NQ : (s + 1) * NQ] = (
            results[core]["out"].transpose(1, 0, 2).reshape(C, NQ)
        )
    return out.reshape(B, C, H, W)


_cache = threading.Lock()
_program = None


def get_program():
    global _program
    with _cache:
        if _program is None:
            _program = build_program()
    return _program


def run(inputs, **kwargs):
    nc = get_program()
    in_maps = prep_inputs(inputs)
    res = run_bass_kernel_spmd(nc, in_maps, core_ids=list(range(8)), **kwargs)
    return assemble_output(res.results), res


def kernel(**inputs):
    out, _ = run(inputs)
    return out
